# revision 8
# baseline (speedup 1.0000x reference)
import math
from functools import partial

import numpy as np
import jax
import jax.numpy as jnp

# Problem: nn_CGABlock_38087769981516
# Hardcoded shapes (grader calls kernel(**inputs) in a bare directory).
B, C_IN, C_OUT, V = 512, 64, 64, 25
MID = C_IN // 8          # 8
INTER = C_OUT // 2       # 32
BN_EPS = 1e-5
N_CORES = 8
BS = B // N_CORES        # 64 samples per core

_PREC = jax.lax.Precision.HIGHEST
_RSQV = 1.0 / math.sqrt(V)

# The monolithic graph trips a PGTiling internal compiler error
# (NCC_IPCC901), so the block is split into small pmapped stages; each
# stage is a simple graph the tensorizer handles. Intermediates stay
# device-resident between stages.


def _s1_proj(x, w1, b1, w2, b2, w3, b3, dw):
    # x: (BS, C_IN, V)
    x1 = jnp.matmul(w1[None], x, precision=_PREC) + b1[:, None]
    x2 = jnp.matmul(w2[None], x, precision=_PREC) + b2[:, None]
    x3 = jnp.matmul(w3[None], x, precision=_PREC) + b3[:, None]
    p = dw[:, 0][None, :, None]
    q = dw[:, 1][None, :, None]
    e1 = p * x1 + q * x2
    e2 = p * x2 + q * x1
    return x1, x2, x3, e1, e2


def _s2_adyn(e1, e2, db):
    # A_dyn[g,u,v] = tanh(e1[g,u] - e2[g,v] + db[g]) ; exact rank-1 form of
    # the grouped pairwise-diff conv in the reference.
    d = e1[:, :, :, None] - e2[:, :, None, :] + db[None, :, None, None]
    return jnp.tanh(d).reshape(-1, MID, V * V)


def _s2b_mix(A_dyn, edge_w, edge_b):
    out = jnp.matmul(edge_w[None], A_dyn, precision=_PREC)
    return out + edge_b[None, :, None]


def _s3_att(x1, x2):
    prod = x1[:, :, :, None] * x2[:, :, None, :]
    return jnp.tanh(prod * _RSQV).reshape(-1, MID, V * V)


def _s3b_mix(att, att_w, att_b):
    out = jnp.matmul(att_w[None], att, precision=_PREC)
    return out + att_b[None, :, None]


def _s4_xatt(x3, att_m):
    # x_att[b,c,v] = sum_u x3[b,c,u] * att_m[b,c,u,v]
    bs = x3.shape[0]
    return jnp.matmul(x3.reshape(bs * C_OUT, 1, V),
                      att_m.reshape(bs * C_OUT, V, V),
                      precision=_PREC).reshape(bs, C_OUT, V)


def _s5_xgcn(A_mix, x3, A_static, alpha):
    A_out = A_static.reshape(1, 1, V * V) + alpha * A_mix
    bs = x3.shape[0]
    return jnp.matmul(A_out.reshape(bs * C_OUT, V, V),
                      x3.reshape(bs * C_OUT, V, 1),
                      precision=_PREC).reshape(bs, C_OUT, V)


def _s6_final(x, x_att, x_gcn0, cc1_w, cc1_b, bn_g, bn_b, bn_m, bn_v,
              cc2_w, cc2_b, cs_w, cs_b):
    xm = x_att.mean(-1, keepdims=True)
    h = jnp.matmul(cc1_w[None], xm, precision=_PREC) + cc1_b[:, None]
    h = (h - bn_m[:, None]) * (bn_g / jnp.sqrt(bn_v + BN_EPS))[:, None] \
        + bn_b[:, None]
    h = jax.nn.gelu(h, approximate=False)
    c_att = jax.nn.sigmoid(
        jnp.matmul(cc2_w[None], h, precision=_PREC) + cc2_b[:, None])
    x_gcn = x_gcn0 * c_att
    s_att = jax.nn.sigmoid(
        jnp.matmul(cs_w[None], x_gcn, precision=_PREC) + cs_b[:, None])
    return x_gcn + x_att * s_att + x


_stages = None


def _get_stages():
    global _stages
    if _stages is None:
        devs = jax.devices()[:N_CORES]
        pm = lambda f, nrep: jax.pmap(
            f, in_axes=tuple([0] * (f.__code__.co_argcount - nrep)
                             + [None] * nrep), devices=devs)
        _stages = {
            's1': pm(_s1_proj, 7),
            's2': pm(_s2_adyn, 1),
            's2b': pm(_s2b_mix, 2),
            's3': pm(_s3_att, 0),
            's3b': pm(_s3b_mix, 2),
            's4': pm(_s4_xatt, 0),
            's5': pm(_s5_xgcn, 2),
            's6': pm(_s6_final, 10),
        }
    return _stages


def kernel(**inputs):
    st = _get_stages()
    f32 = np.float32
    g = {k: np.asarray(v, dtype=f32) for k, v in inputs.items()}
    xs = g['x'].reshape(N_CORES, BS, C_IN, V)

    x1, x2, x3, e1, e2 = st['s1'](xs, g['w1'], g['b1'], g['w2'], g['b2'],
                                  g['w3'], g['b3'], g['diff_w'])
    A_dyn = st['s2'](e1, e2, g['diff_b'])
    A_mix = st['s2b'](A_dyn, g['edge_w'], g['edge_b'])
    att = st['s3'](x1, x2)
    att_m = st['s3b'](att, g['att_w'], g['att_b'])
    att_m = att_m.reshape(N_CORES, BS, C_OUT, V, V)
    x_att = st['s4'](x3, att_m)
    x_gcn0 = st['s5'](A_mix, x3, g['A_static'], g['alpha'])
    out = st['s6'](xs, x_att, x_gcn0, g['cc1_w'], g['cc1_b'], g['bn_g'],
                   g['bn_b'], g['bn_m'], g['bn_v'], g['cc2_w'], g['cc2_b'],
                   g['cs_w'], g['cs_b'])
    return np.asarray(out).reshape(B, C_OUT, V).astype(np.float32)


# revision 11
# speedup vs baseline: 1.1312x; 1.1312x over previous
import math
from functools import partial

import numpy as np
import jax
import jax.numpy as jnp

# Problem: nn_CGABlock_38087769981516
# Hardcoded shapes (grader calls kernel(**inputs) in a bare directory).
B, C_IN, C_OUT, V = 512, 64, 64, 25
MID = C_IN // 8          # 8
INTER = C_OUT // 2       # 32
BN_EPS = 1e-5
N_CORES = 8
BS = B // N_CORES        # 64 samples per core

_PREC = jax.lax.Precision.HIGHEST
_RSQV = 1.0 / math.sqrt(V)

# The monolithic graph trips a PGTiling internal compiler error
# (NCC_IPCC901), so the block is split into small pmapped stages; each
# stage is a simple graph the tensorizer handles. Intermediates stay
# device-resident between stages.


def _s1_proj(x, w1, b1, w2, b2, w3, b3, dw):
    # x: (BS, C_IN, V)
    x1 = jnp.matmul(w1[None], x, precision=_PREC) + b1[:, None]
    x2 = jnp.matmul(w2[None], x, precision=_PREC) + b2[:, None]
    x3 = jnp.matmul(w3[None], x, precision=_PREC) + b3[:, None]
    # Grouped conv pairing: concat([d1,d2],ch).reshape(MID,2,..) gives group
    # g the channels (2g, 2g+1) of the *concatenated* tensor — g<4 reads two
    # d1 channels, g>=4 two d2 channels. Exact rank-1 form:
    #   A_dyn[g,u,v] = tanh(f1[g,u] - f2[g,v] + db[g])
    x1r = x1.reshape(-1, MID // 2, 2, V)
    x2r = x2.reshape(-1, MID // 2, 2, V)
    dwa = dw[:MID // 2].reshape(1, MID // 2, 2, 1)
    dwb = dw[MID // 2:].reshape(1, MID // 2, 2, 1)
    f1 = jnp.concatenate([(x1r * dwa).sum(2), (x2r * dwb).sum(2)], axis=1)
    f2 = jnp.concatenate([(x2r * dwa).sum(2), (x1r * dwb).sum(2)], axis=1)
    return x1, x2, x3, f1, f2


def _s2_adyn(e1, e2, db):
    # A_dyn[g,u,v] = tanh(e1[g,u] - e2[g,v] + db[g]) ; exact rank-1 form of
    # the grouped pairwise-diff conv in the reference.
    d = e1[:, :, :, None] - e2[:, :, None, :] + db[None, :, None, None]
    return jnp.tanh(d).reshape(-1, MID, V * V)


def _s2b_mix(A_dyn, edge_w, edge_b):
    out = jnp.matmul(edge_w[None], A_dyn, precision=_PREC)
    return out + edge_b[None, :, None]


def _s3_att(x1, x2):
    prod = x1[:, :, :, None] * x2[:, :, None, :]
    return jnp.tanh(prod * _RSQV).reshape(-1, MID, V * V)


def _s3b_mix(att, att_w, att_b):
    out = jnp.matmul(att_w[None], att, precision=_PREC)
    out = out + att_b[None, :, None]
    return out.reshape(-1, C_OUT, V, V)


def _s4_xatt(x3, att_m):
    # x_att[b,c,v] = sum_u x3[b,c,u] * att_m[b,c,u,v]
    bs = x3.shape[0]
    return jnp.matmul(x3.reshape(bs * C_OUT, 1, V),
                      att_m.reshape(bs * C_OUT, V, V),
                      precision=_PREC).reshape(bs, C_OUT, V)


def _s5_xgcn(A_mix, x3, A_static, alpha):
    A_out = A_static.reshape(1, 1, V * V) + alpha * A_mix
    bs = x3.shape[0]
    return jnp.matmul(A_out.reshape(bs * C_OUT, V, V),
                      x3.reshape(bs * C_OUT, V, 1),
                      precision=_PREC).reshape(bs, C_OUT, V)


def _s6_final(x, x_att, x_gcn0, cc1_w, cc1_b, bn_g, bn_b, bn_m, bn_v,
              cc2_w, cc2_b, cs_w, cs_b):
    xm = x_att.mean(-1, keepdims=True)
    h = jnp.matmul(cc1_w[None], xm, precision=_PREC) + cc1_b[:, None]
    h = (h - bn_m[:, None]) * (bn_g / jnp.sqrt(bn_v + BN_EPS))[:, None] \
        + bn_b[:, None]
    h = jax.nn.gelu(h, approximate=False)
    c_att = jax.nn.sigmoid(
        jnp.matmul(cc2_w[None], h, precision=_PREC) + cc2_b[:, None])
    x_gcn = x_gcn0 * c_att
    s_att = jax.nn.sigmoid(
        jnp.matmul(cs_w[None], x_gcn, precision=_PREC) + cs_b[:, None])
    return x_gcn + x_att * s_att + x


_stages = None


def _get_stages():
    global _stages
    if _stages is None:
        devs = jax.devices()[:N_CORES]
        pm = lambda f, nrep: jax.pmap(
            f, in_axes=tuple([0] * (f.__code__.co_argcount - nrep)
                             + [None] * nrep), devices=devs)
        _stages = {
            's1': pm(_s1_proj, 7),
            's2': pm(_s2_adyn, 1),
            's2b': pm(_s2b_mix, 2),
            's3': pm(_s3_att, 0),
            's3b': pm(_s3b_mix, 2),
            's4': pm(_s4_xatt, 0),
            's5': pm(_s5_xgcn, 2),
            's6': pm(_s6_final, 10),
        }
    return _stages


def kernel(**inputs):
    st = _get_stages()
    f32 = np.float32
    g = {k: np.asarray(v, dtype=f32) for k, v in inputs.items()}
    xs = g['x'].reshape(N_CORES, BS, C_IN, V)

    x1, x2, x3, e1, e2 = st['s1'](xs, g['w1'], g['b1'], g['w2'], g['b2'],
                                  g['w3'], g['b3'], g['diff_w'])
    A_dyn = st['s2'](e1, e2, g['diff_b'])
    A_mix = st['s2b'](A_dyn, g['edge_w'], g['edge_b'])
    att = st['s3'](x1, x2)
    att_m = st['s3b'](att, g['att_w'], g['att_b'])
    x_att = st['s4'](x3, att_m)
    x_gcn0 = st['s5'](A_mix, x3, g['A_static'], g['alpha'])
    out = st['s6'](xs, x_att, x_gcn0, g['cc1_w'], g['cc1_b'], g['bn_g'],
                   g['bn_b'], g['bn_m'], g['bn_v'], g['cc2_w'], g['cc2_b'],
                   g['cs_w'], g['cs_b'])
    return np.asarray(out).reshape(B, C_OUT, V).astype(np.float32)
